# revision 5
# baseline (speedup 1.0000x reference)
"""MoE-routed autoencoder (4 experts, 1024->512->128->512->1024) on 8 TRN2 cores.

Strategy:
- Host: sort atoms by expert symbol, deal each expert's atoms evenly across the
  8 cores, pad per-(core,expert) groups to a common per-expert capacity so one
  SPMD program serves all cores. Only the routed expert runs per atom (4x less
  compute than dense dispatch).
- Device: activations live in transposed layout [feat, atoms] so every layer is
  out[M=feat_tile, N=atom_cols] = W[K,M].T @ act[K, N] on the tensor engine
  (f16 operands, fp32 PSUM accumulation). ReLU fuses into the PSUM->SBUF
  eviction on the scalar engine.
- L1's K=1024 contraction runs as two 4-deep PSUM bursts per bank interleaved
  across the 4 m-chunks: long 8-deep bursts that cycle PSUM banks drop the PE
  clock from 2.4 to 2.0 GHz on this part; <=4-deep bursts keep 2.4 GHz.
- DMA packet-rate discipline: the DGE rings dispatch ~100 packets/us and each
  packet is at most the SBUF per-partition run, so every transfer keeps
  per-partition runs >= 2.7KB: x and w1 load as k-half tiles [128, 4*T] /
  [128, 4*H1], w2+w3 merge into one [128, 1024] tile, y stores as two m-half
  tiles [128, 4*T] per tile. x/w1 ride the sync HWDGE ring; w23/w4/y ride the
  gpsimd SWDGE ring.
- Y is stored as f16 (halves writeback traffic; host upconverts).
- Host: scatter the routed outputs back to the original atom order.
"""

import math

import numpy as np

N_CORES = 8


def _round_fp32r(a: np.ndarray) -> np.ndarray:
    """Round-to-nearest-even fp32 -> fp32r (11-bit mantissa), bitwise."""
    u = np.ascontiguousarray(a, dtype=np.float32).view(np.uint32)
    lsb = (u >> 12) & np.uint32(1)
    r = (u + np.uint32(0x7FF) + lsb) & np.uint32(0xFFFFF000)
    return r.view(np.float32)


_PROGRAM_CACHE: dict = {}

# matmul operand precision: "f16" (10-bit mantissa, half the DMA/LDW cost)
# or "f32r" (11-bit mantissa, fp32-sized operands). PSUM accumulation is
# fp32 either way.
_MODE = "f16"
# store Y as f16 on device (host converts back to f32)
_Y_F16 = True
_WARMUP = 24

# test-harness knobs: when _TRACE is set, the SPMD launch requests an NTFF
# profile and the BassKernelResults lands in _LAST["res"].
_TRACE = False
_LAST: dict = {}


def _plan(dims, tiles):
    """Flat-buffer offsets for the tile-order X / Y layouts.

    tiles[e] = (col_offset, [T_0, T_1, ...]) per-tile widths."""
    D_IN, H1, LAT, D_OUT, E, C_tot = dims
    KC1 = D_IN // 128
    MC4 = D_OUT // 128
    xoff, yoff, seq = 0, 0, []
    for e in range(E):
        off, Ts = tiles[e]
        co = off
        for t, T in enumerate(Ts):
            seq.append((e, t, co, T, xoff, yoff))
            co += T
            xoff += 128 * KC1 * T
            yoff += 128 * MC4 * T
    return seq, xoff, yoff


def _build_program(dims, tiles, use_bias, n_bias_cols, mode):
    import concourse.bass as bass  # noqa: F401
    import concourse.tile as tile
    from concourse import bacc, mybir

    D_IN, H1, LAT, D_OUT, E, C_tot = dims
    f32 = mybir.dt.float32
    mdt = mybir.dt.float16 if mode == "f16" else mybir.dt.float32r
    ydt = mybir.dt.float16 if _Y_F16 else f32
    RELU = mybir.ActivationFunctionType.Relu
    IDENT = mybir.ActivationFunctionType.Identity

    KC1 = D_IN // 128   # 8  k-chunks layer 1
    MC1 = H1 // 128     # 4  m-chunks layer 1
    KC2 = H1 // 128     # 4
    MC2 = LAT // 128    # 1
    KC3 = LAT // 128    # 1
    MC3 = H1 // 128     # 4
    KC4 = H1 // 128     # 4
    MC4 = D_OUT // 128  # 8
    KH = KC1 // 2       # 4  k-chunks per half (L1 phase)
    MH = MC4 // 2       # 4  m-chunks per y half

    seq, x_total, y_total = _plan(dims, tiles)

    nc = bacc.Bacc("TRN2", target_bir_lowering=False, debug=False,
                   num_devices=N_CORES)
    xt = nc.dram_tensor("xt", [x_total], mdt, kind="ExternalInput").ap()
    w1 = nc.dram_tensor("w1", [E, 128, KC1 * H1], mdt,
                        kind="ExternalInput").ap()
    w23 = nc.dram_tensor("w23", [E, 128, KC2 * LAT + KC3 * H1], mdt,
                         kind="ExternalInput").ap()
    w4 = nc.dram_tensor("w4", [E, 128, KC4 * D_OUT], mdt,
                        kind="ExternalInput").ap()
    if use_bias:
        bias = nc.dram_tensor("bias", [128, n_bias_cols], f32,
                              kind="ExternalInput").ap()
    yt = nc.dram_tensor("yt", [y_total], ydt, kind="ExternalOutput").ap()

    with tile.TileContext(nc) as tc:
        with (
            tc.tile_pool(name="wp", bufs=2) as wp,
            tc.tile_pool(name="xp", bufs=3) as xp,
            tc.tile_pool(name="hp", bufs=3) as hp,
            tc.tile_pool(name="zp", bufs=3) as zp,
            tc.tile_pool(name="dp", bufs=3) as dp,
            tc.tile_pool(name="yp", bufs=3) as yp,
            tc.tile_pool(name="bp", bufs=1) as bp,
            tc.tile_pool(name="pp1", bufs=4, space="PSUM") as pp1,
            tc.tile_pool(name="pp23", bufs=2, space="PSUM") as pp23,
            tc.tile_pool(name="ppb", bufs=2, space="PSUM") as ppb,
        ):
            if use_bias:
                btile = bp.tile([128, n_bias_cols], f32)
                nc.gpsimd.dma_start(btile[:], bias[:])
                bias_col = [0]

                def next_bias():
                    c = bias_col[0]
                    bias_col[0] += 1
                    return btile[:, c:c + 1]

            def evict(out_ap, ps_ap, relu, on_vector=False):
                if relu:
                    b = next_bias() if use_bias else 0.0
                    nc.scalar.activation(out_ap, ps_ap, RELU, bias=b)
                elif use_bias:
                    nc.scalar.activation(out_ap, ps_ap, IDENT, bias=next_bias())
                elif on_vector:
                    nc.vector.tensor_copy(out_ap, ps_ap)
                else:
                    nc.scalar.activation(out_ap, ps_ap,
                                         mybir.ActivationFunctionType.Copy)

            # PE warmup: dependency-free matmuls on a memset tile release the
            # HAM clock gate while the prologue DMA is in flight.
            warm = bp.tile([128, 128], mdt, tag="warm")
            nc.vector.memset(warm[:], 0.0)
            wps = ppb.tile([128, 128], f32, tag="ps")
            for _ in range(_WARMUP):
                nc.tensor.matmul(wps[:], warm[:], warm[:],
                                 start=True, stop=True)

            wtiles = {}

            def load_w1_half(e, h):
                w1h = wp.tile([128, KH * H1], mdt, tag=f"w1h{h}",
                              name=f"w1h{h}")
                nc.sync.dma_start(w1h[:],
                                  w1[e][:, h * KH * H1:(h + 1) * KH * H1])
                return w1h

            def load_rest(e):
                # w23 + w4 ride the gpsimd SWDGE ring
                w23t = wp.tile([128, KC2 * LAT + KC3 * H1], mdt, tag="w23")
                w4t = wp.tile([128, KC4 * D_OUT], mdt, tag="w4")
                nc.gpsimd.dma_start(w23t[:], w23[e])
                nc.gpsimd.dma_start(w4t[:], w4[e])
                return w23t, w4t

            def load_x_half(si, h):
                e, t, co, T, xo, yo = seq[si]
                half = 128 * KH * T
                xh = xp.tile([128, KH * T], mdt, tag=f"xh{h}", name=f"xh{h}")
                nc.sync.dma_start(
                    xh[:],
                    xt[xo + h * half:xo + (h + 1) * half]
                    .rearrange("(p f) -> p f", p=128))
                return xh

            xpre = {}
            cur_e = 0
            for si, (e, t, co, T, xo, yo) in enumerate(seq):
                if e != cur_e:
                    wtiles.pop(cur_e)
                    cur_e = e

                if si == 0:
                    # prologue in consumption order on the sync ring:
                    # w1 half 0, x half 0, w1 half 1, x half 1
                    w1h0 = load_w1_half(0, 0)
                    xh0 = load_x_half(0, 0)
                    w1h1 = load_w1_half(0, 1)
                    xh1 = load_x_half(0, 1)
                    xhs = [xh0, xh1]
                    w23t, w4t = load_rest(0)
                    wtiles[0] = ([w1h0, w1h1], w23t, w4t)
                else:
                    xhs = xpre.pop(si, None)
                    if xhs is None:
                        xhs = [load_x_half(si, 0), load_x_half(si, 1)]
                if si + 1 < len(seq) and si + 1 not in xpre:
                    xpre[si + 1] = [load_x_half(si + 1, 0),
                                    load_x_half(si + 1, 1)]
                if t == 1 and e + 1 < E:
                    w1hs_n = [load_w1_half(e + 1, 0), load_w1_half(e + 1, 1)]
                    w23t_n, w4t_n = load_rest(e + 1)
                    wtiles[e + 1] = (w1hs_n, w23t_n, w4t_n)
                w1hs, w23t, w4t = wtiles[e]

                # L1: h[H1, T] = relu(W1.T @ x), two 4-deep bursts per bank
                htile = hp.tile([128, MC1 * T], mdt, tag="h")
                pss = []
                for m in range(MC1):
                    ps_m = pp1.tile([128, T], f32, tag="ps", name=f"ps{m}")
                    pss.append(ps_m)
                for ph in range(2):
                    for m in range(MC1):
                        for kl in range(KH):
                            nc.tensor.matmul(
                                pss[m][:],
                                w1hs[ph][:, kl * H1 + m * 128:
                                         kl * H1 + (m + 1) * 128],
                                xhs[ph][:, kl * T:(kl + 1) * T],
                                start=(ph == 0 and kl == 0),
                                stop=(ph == 1 and kl == KH - 1))
                        if ph == 1:
                            evict(htile[:, m * T:(m + 1) * T], pss[m][:],
                                  relu=True)

                # L2: z[LAT, T] = relu(W2.T @ h)   (w2 = w23 cols [0, 512))
                ztile = zp.tile([128, MC2 * T], mdt, tag="z")
                for m in range(MC2):
                    ps = pp23.tile([128, T], f32, tag="ps")
                    for k in range(KC2):
                        nc.tensor.matmul(
                            ps[:],
                            w23t[:, k * LAT + m * 128:k * LAT + (m + 1) * 128],
                            htile[:, k * T:(k + 1) * T],
                            start=(k == 0), stop=(k == KC2 - 1))
                    evict(ztile[:, m * T:(m + 1) * T], ps[:], relu=True)

                # L3: d[H1, T] = relu(W3.T @ z)   (w3 = w23 cols [512, 1024))
                w3off = KC2 * LAT
                dtile = dp.tile([128, MC3 * T], mdt, tag="d")
                for m in range(MC3):
                    ps = pp23.tile([128, T], f32, tag="ps")
                    for k in range(KC3):
                        nc.tensor.matmul(
                            ps[:],
                            w23t[:, w3off + k * H1 + m * 128:
                                 w3off + k * H1 + (m + 1) * 128],
                            ztile[:, k * T:(k + 1) * T],
                            start=(k == 0), stop=(k == KC3 - 1))
                    evict(dtile[:, m * T:(m + 1) * T], ps[:], relu=True)

                # L4: y[D_OUT, T] = W4.T @ d  (no relu); two m-half stores
                ytile = yp.tile([128, MC4 * T], ydt, tag="y")
                yhalf = 128 * MH * T
                for m in range(MC4):
                    ps = ppb.tile([128, T], f32, tag="ps")
                    for k in range(KC4):
                        nc.tensor.matmul(
                            ps[:],
                            w4t[:, k * D_OUT + m * 128:k * D_OUT + (m + 1) * 128],
                            dtile[:, k * T:(k + 1) * T],
                            start=(k == 0), stop=(k == KC4 - 1))
                    evict(ytile[:, m * T:(m + 1) * T], ps[:], relu=False,
                          on_vector=(m % 2 == 0))
                    if m % MH == MH - 1:
                        h = m // MH
                        nc.gpsimd.dma_start(
                            yt[yo + h * yhalf:yo + (h + 1) * yhalf]
                            .rearrange("(p f) -> p f", p=128),
                            ytile[:, h * MH * T:(h + 1) * MH * T])

    nc.compile()
    return nc


def kernel(**inputs) -> np.ndarray:
    from concourse.bass_utils import run_bass_kernel_spmd

    X = np.ascontiguousarray(inputs["X"], dtype=np.float32)
    sym_ids = np.asarray(inputs["sym_ids"]).astype(np.int64).ravel()
    We = [inputs["We1"], inputs["We2"], inputs["Wd1"], inputs["Wd2"]]
    be = [np.asarray(inputs["be1"], dtype=np.float32),
          np.asarray(inputs["be2"], dtype=np.float32),
          np.asarray(inputs["bd1"], dtype=np.float32),
          np.asarray(inputs["bd2"], dtype=np.float32)]

    N, D_IN = X.shape
    E, _, H1 = We[0].shape
    LAT = We[1].shape[2]
    D_OUT = We[3].shape[2]
    KC1 = D_IN // 128
    MC4 = D_OUT // 128
    KH = KC1 // 2
    MH = MC4 // 2
    use_bias = any(np.any(b) for b in be)

    # ---- host routing: per-expert, per-core index assignment ----
    core_idx = [[None] * E for _ in range(N_CORES)]
    C_e = [0] * E
    for e in range(E):
        idx = np.flatnonzero(sym_ids == e)
        n = len(idx)
        base, rem = divmod(n, N_CORES)
        s = 0
        for c in range(N_CORES):
            cnt = base + (1 if c < rem else 0)
            core_idx[c][e] = idx[s:s + cnt]
            s += cnt
        C_e[e] = base + (1 if rem else 0)

    # per-expert column tiling: tiles of width <=512, multiples of 8
    tiles = []
    off = 0
    for e in range(E):
        ce = max(C_e[e], 1)
        nt = max(1, math.ceil(ce / 512))
        T = -(-math.ceil(ce / nt) // 8) * 8
        Ts = [T] * nt
        tiles.append((off, tuple(Ts)))
        off += sum(Ts)
    C_tot = off

    # ---- build / fetch compiled program ----
    dims = (D_IN, H1, LAT, D_OUT, E, C_tot)
    n_bias_cols = E * (H1 + LAT + H1 + D_OUT) // 128
    key = (dims, tuple(tiles), use_bias, _MODE, _Y_F16)
    nc = _PROGRAM_CACHE.get(key)
    if nc is None:
        nc = _build_program(dims, tiles, use_bias, n_bias_cols, _MODE)
        _PROGRAM_CACHE[key] = nc

    # ---- prepare inputs ----
    if _MODE == "f16":
        rnd = lambda a: np.ascontiguousarray(a, dtype=np.float32).astype(
            np.float16)
        mm_np = np.float16
    else:
        rnd = _round_fp32r
        mm_np = np.float32
    XrT = np.ascontiguousarray(rnd(X).T)                     # [D_IN, N]
    XrT_z = np.concatenate(
        [XrT, np.zeros((D_IN, 1), mm_np)], axis=1)           # pad col = N

    # weights in device layout: [E, 128, kc*m] (k-chunk-major columns)
    def wdev(w, kc, mw):
        return np.ascontiguousarray(
            rnd(w).reshape(E, kc, 128, mw).transpose(0, 2, 1, 3)
            .reshape(E, 128, kc * mw))

    W1d = wdev(We[0], KC1, H1)
    W23d = np.ascontiguousarray(np.concatenate(
        [wdev(We[1], H1 // 128, LAT), wdev(We[2], LAT // 128, H1)], axis=2))
    W4d = wdev(We[3], H1 // 128, D_OUT)

    seq, x_total, y_total = _plan(dims, tiles)

    bias_h = None
    if use_bias:
        bias_h = np.zeros((128, n_bias_cols), np.float32)
        col = 0
        for e in range(E):
            for b in (be[0][e], be[1][e], be[2][e], be[3][e]):
                for mch in range(len(b) // 128):
                    bias_h[:, col] = b[mch * 128:(mch + 1) * 128]
                    col += 1

    perms = []
    in_maps = []
    for c in range(N_CORES):
        perm = np.full(C_tot, N, dtype=np.int64)
        for e in range(E):
            o = tiles[e][0]
            idx = core_idx[c][e]
            perm[o:o + len(idx)] = idx
        perms.append(perm)
        g3 = XrT_z[:, perm].reshape(KC1, 128, C_tot)
        xflat = np.empty(x_total, dtype=mm_np)
        for e, t, co, T, xo, yo in seq:
            # two k-halves per tile, each partition-major [128, KH*T] so a
            # half is one contiguous DMA with 2*KH*T-byte per-partition runs
            half = 128 * KH * T
            xflat[xo:xo + half] = (
                g3[:KH, :, co:co + T].transpose(1, 0, 2).reshape(-1))
            xflat[xo + half:xo + 2 * half] = (
                g3[KH:, :, co:co + T].transpose(1, 0, 2).reshape(-1))
        m = {"xt": xflat, "w1": W1d, "w23": W23d, "w4": W4d}
        if use_bias:
            m["bias"] = bias_h
        in_maps.append(m)

    res = run_bass_kernel_spmd(nc, in_maps, core_ids=list(range(N_CORES)),
                               trace=_TRACE)
    _LAST["res"] = res

    # ---- unshard ----
    Y = np.empty((N, D_OUT), dtype=np.float32)
    for c in range(N_CORES):
        yflat = np.asarray(res.results[c]["yt"], dtype=np.float32)
        ytc = np.empty((D_OUT, C_tot), dtype=np.float32)
        for e, t, co, T, xo, yo in seq:
            yhalf = 128 * MH * T
            for h in range(2):
                blk = (yflat[yo + h * yhalf:yo + (h + 1) * yhalf]
                       .reshape(128, MH, T).transpose(1, 0, 2)
                       .reshape(MH * 128, T))
                ytc[h * MH * 128:(h + 1) * MH * 128, co:co + T] = blk
        perm = perms[c]
        valid = perm != N
        Y[perm[valid]] = ytc.T[valid]
    return Y


# revision 6
# speedup vs baseline: 1.1987x; 1.1987x over previous
"""MoE-routed autoencoder (4 experts, 1024->512->128->512->1024) on 8 TRN2 cores.

Strategy:
- Host: sort atoms by expert symbol, deal each expert's atoms evenly across the
  8 cores, pad per-(core,expert) groups to a common per-expert capacity so one
  SPMD program serves all cores. Only the routed expert runs per atom (4x less
  compute than dense dispatch).
- Device: activations live in transposed layout [feat, atoms] so every layer is
  out[M=feat_tile, N=atom_cols] = W[K,M].T @ act[K, N] on the tensor engine
  (f16 operands, fp32 PSUM accumulation). ReLU fuses into the PSUM->SBUF
  eviction on the scalar engine.
- L1's K=1024 contraction runs as two 4-deep PSUM bursts per bank interleaved
  across the 4 m-chunks; 8-deep bursts that cycle PSUM banks (and sustained
  high-bandwidth DMA) drop the PE clock from 2.4 to 2.0 GHz on this part.
- DMA pacing: steady-state x/w1 loads are per-k-chunk tiles whose small
  (0.7-1KB) per-partition runs rate-limit the sync HWDGE ring to ~80GB/s --
  just-in-time for the PE while keeping chip DMA below the clock-throttle
  threshold. Only the expert-0/tile-0 prologue uses big-packet half tiles so
  the first matmul starts ~12us in. y rides the scalar HWDGE ring (hardware
  descriptor generation -- the gpsimd SWDGE ucode was the old tail
  bottleneck) as two [128, 4T] half stores per tile; w23/w4/w1-prefetch ride
  the gpsimd SWDGE ring.
- Y is stored as f16 (halves writeback traffic; host upconverts).
- Host: scatter the routed outputs back to the original atom order.
"""

import math

import numpy as np

N_CORES = 8


def _round_fp32r(a: np.ndarray) -> np.ndarray:
    """Round-to-nearest-even fp32 -> fp32r (11-bit mantissa), bitwise."""
    u = np.ascontiguousarray(a, dtype=np.float32).view(np.uint32)
    lsb = (u >> 12) & np.uint32(1)
    r = (u + np.uint32(0x7FF) + lsb) & np.uint32(0xFFFFF000)
    return r.view(np.float32)


_PROGRAM_CACHE: dict = {}

_MODE = "f16"
_Y_F16 = True
_WARMUP = 24

_TRACE = False
_LAST: dict = {}


def _plan(dims, tiles):
    """Flat-buffer offsets for the tile-order X / Y layouts."""
    D_IN, H1, LAT, D_OUT, E, C_tot = dims
    KC1 = D_IN // 128
    MC4 = D_OUT // 128
    xoff, yoff, seq = 0, 0, []
    for e in range(E):
        off, Ts = tiles[e]
        co = off
        for t, T in enumerate(Ts):
            seq.append((e, t, co, T, xoff, yoff))
            co += T
            xoff += 128 * KC1 * T
            yoff += 128 * MC4 * T
    return seq, xoff, yoff


def _build_program(dims, tiles, use_bias, n_bias_cols, mode):
    import concourse.bass as bass  # noqa: F401
    import concourse.tile as tile
    from concourse import bacc, mybir

    D_IN, H1, LAT, D_OUT, E, C_tot = dims
    f32 = mybir.dt.float32
    mdt = mybir.dt.float16 if mode == "f16" else mybir.dt.float32r
    ydt = mybir.dt.float16 if _Y_F16 else f32
    RELU = mybir.ActivationFunctionType.Relu
    IDENT = mybir.ActivationFunctionType.Identity

    KC1 = D_IN // 128   # 8
    MC1 = H1 // 128     # 4
    KC2 = H1 // 128     # 4
    MC2 = LAT // 128    # 1
    KC3 = LAT // 128    # 1
    MC3 = H1 // 128     # 4
    KC4 = H1 // 128     # 4
    MC4 = D_OUT // 128  # 8
    KH = KC1 // 2       # 4
    MH = MC4 // 2       # 4

    seq, x_total, y_total = _plan(dims, tiles)

    nc = bacc.Bacc("TRN2", target_bir_lowering=False, debug=False,
                   num_devices=N_CORES)
    xt = nc.dram_tensor("xt", [x_total], mdt, kind="ExternalInput").ap()
    w1 = nc.dram_tensor("w1", [E, 128, KC1 * H1], mdt,
                        kind="ExternalInput").ap()
    w23 = nc.dram_tensor("w23", [E, 128, KC2 * LAT + KC3 * H1], mdt,
                         kind="ExternalInput").ap()
    w4 = nc.dram_tensor("w4", [E, 128, KC4 * D_OUT], mdt,
                        kind="ExternalInput").ap()
    if use_bias:
        bias = nc.dram_tensor("bias", [128, n_bias_cols], f32,
                              kind="ExternalInput").ap()
    yt = nc.dram_tensor("yt", [y_total], ydt, kind="ExternalOutput").ap()

    with tile.TileContext(nc) as tc:
        with (
            tc.tile_pool(name="wp", bufs=2) as wp,
            tc.tile_pool(name="xp", bufs=3) as xp,
            tc.tile_pool(name="hp", bufs=3) as hp,
            tc.tile_pool(name="zp", bufs=3) as zp,
            tc.tile_pool(name="dp", bufs=3) as dp,
            tc.tile_pool(name="yp", bufs=3) as yp,
            tc.tile_pool(name="bp", bufs=1) as bp,
            tc.tile_pool(name="pp1", bufs=4, space="PSUM") as pp1,
            tc.tile_pool(name="pp23", bufs=2, space="PSUM") as pp23,
            tc.tile_pool(name="ppb", bufs=2, space="PSUM") as ppb,
        ):
            if use_bias:
                btile = bp.tile([128, n_bias_cols], f32)
                nc.gpsimd.dma_start(btile[:], bias[:])
                bias_col = [0]

                def next_bias():
                    c = bias_col[0]
                    bias_col[0] += 1
                    return btile[:, c:c + 1]

            def evict(out_ap, ps_ap, relu, on_vector=False):
                if relu:
                    b = next_bias() if use_bias else 0.0
                    nc.scalar.activation(out_ap, ps_ap, RELU, bias=b)
                elif use_bias:
                    nc.scalar.activation(out_ap, ps_ap, IDENT, bias=next_bias())
                elif on_vector:
                    nc.vector.tensor_copy(out_ap, ps_ap)
                else:
                    nc.scalar.activation(out_ap, ps_ap,
                                         mybir.ActivationFunctionType.Copy)

            # PE warmup while the prologue DMA is in flight
            warm = bp.tile([128, 128], mdt, tag="warm")
            nc.vector.memset(warm[:], 0.0)
            wps = ppb.tile([128, 128], f32, tag="ps")
            for _ in range(_WARMUP):
                nc.tensor.matmul(wps[:], warm[:], warm[:],
                                 start=True, stop=True)

            wtiles = {}

            # --- weight loaders ---
            def load_w1_e0():
                """expert-0 w1 as two big-packet halves on the sync ring."""
                halves = []
                for h in range(2):
                    w1h = wp.tile([128, KH * H1], mdt, tag=f"w1h{h}",
                                  name=f"w1h{h}")
                    nc.sync.dma_start(
                        w1h[:], w1[0][:, h * KH * H1:(h + 1) * KH * H1])
                    halves.append(w1h)

                def acc(k, m):
                    return halves[k // KH][:, (k % KH) * H1 + m * 128:
                                           (k % KH) * H1 + (m + 1) * 128]
                return acc

            def load_w1_chunks(e):
                """later experts: per-k chunks on the gpsimd ring."""
                cs = []
                for k in range(KC1):
                    w1k = wp.tile([128, H1], mdt, tag=f"w1k{k}",
                                  name=f"w1k{k}")
                    nc.gpsimd.dma_start(w1k[:],
                                        w1[e][:, k * H1:(k + 1) * H1])
                    cs.append(w1k)

                def acc(k, m):
                    return cs[k][:, m * 128:(m + 1) * 128]
                return acc

            def load_rest(e):
                w23t = wp.tile([128, KC2 * LAT + KC3 * H1], mdt, tag="w23")
                w4t = wp.tile([128, KC4 * D_OUT], mdt, tag="w4")
                nc.gpsimd.dma_start(w23t[:], w23[e])
                nc.gpsimd.dma_start(w4t[:], w4[e])
                return w23t, w4t

            # --- x loaders ---
            def load_x_halves(si):
                """tile 0: two big-packet halves (partition-major layout)."""
                e, t, co, T, xo, yo = seq[si]
                half = 128 * KH * T
                hs = []
                for h in range(2):
                    xh = xp.tile([128, KH * T], mdt, tag=f"xh{h}",
                                 name=f"xh{h}")
                    nc.sync.dma_start(
                        xh[:],
                        xt[xo + h * half:xo + (h + 1) * half]
                        .rearrange("(p f) -> p f", p=128))
                    hs.append(xh)

                def acc(k):
                    return hs[k // KH][:, (k % KH) * T:(k % KH + 1) * T]
                return acc

            def load_x_chunks(si):
                """tiles >=1: per-k chunk tiles (k-major layout) whose small
                per-partition runs pace the sync ring."""
                e, t, co, T, xo, yo = seq[si]
                cs = []
                for k in range(KC1):
                    xk = xp.tile([128, T], mdt, tag=f"xk{k}", name=f"xk{k}")
                    nc.sync.dma_start(
                        xk[:],
                        xt[xo + k * 128 * T:xo + (k + 1) * 128 * T]
                        .rearrange("(p f) -> p f", p=128))
                    cs.append(xk)

                def acc(k):
                    return cs[k][:]
                return acc

            xpre = {}
            cur_e = 0
            for si, (e, t, co, T, xo, yo) in enumerate(seq):
                if e != cur_e:
                    wtiles.pop(cur_e)
                    cur_e = e

                if si == 0:
                    w1a = load_w1_e0()
                    xa = load_x_halves(0)
                    w23t, w4t = load_rest(0)
                    wtiles[0] = (w1a, w23t, w4t)
                else:
                    xa = xpre.pop(si, None)
                    if xa is None:
                        xa = load_x_chunks(si)
                if si + 1 < len(seq) and si + 1 not in xpre:
                    xpre[si + 1] = load_x_chunks(si + 1)
                if t == 1 and e + 1 < E:
                    w1a_n = load_w1_chunks(e + 1)
                    w23t_n, w4t_n = load_rest(e + 1)
                    wtiles[e + 1] = (w1a_n, w23t_n, w4t_n)
                w1a, w23t, w4t = wtiles[e]

                # L1: h[H1, T] = relu(W1.T @ x), two 4-deep bursts per bank
                htile = hp.tile([128, MC1 * T], mdt, tag="h")
                pss = []
                for m in range(MC1):
                    ps_m = pp1.tile([128, T], f32, tag="ps", name=f"ps{m}")
                    pss.append(ps_m)
                for ph in range(2):
                    for m in range(MC1):
                        for kl in range(KH):
                            k = ph * KH + kl
                            nc.tensor.matmul(
                                pss[m][:], w1a(k, m), xa(k),
                                start=(ph == 0 and kl == 0),
                                stop=(ph == 1 and kl == KH - 1))
                        if ph == 1:
                            evict(htile[:, m * T:(m + 1) * T], pss[m][:],
                                  relu=True)

                # L2: z[LAT, T] = relu(W2.T @ h)   (w2 = w23 cols [0, 512))
                ztile = zp.tile([128, MC2 * T], mdt, tag="z")
                for m in range(MC2):
                    ps = pp23.tile([128, T], f32, tag="ps")
                    for k in range(KC2):
                        nc.tensor.matmul(
                            ps[:],
                            w23t[:, k * LAT + m * 128:k * LAT + (m + 1) * 128],
                            htile[:, k * T:(k + 1) * T],
                            start=(k == 0), stop=(k == KC2 - 1))
                    evict(ztile[:, m * T:(m + 1) * T], ps[:], relu=True)

                # L3: d[H1, T] = relu(W3.T @ z)   (w3 = w23 cols [512, 1024))
                w3off = KC2 * LAT
                dtile = dp.tile([128, MC3 * T], mdt, tag="d")
                for m in range(MC3):
                    ps = pp23.tile([128, T], f32, tag="ps")
                    for k in range(KC3):
                        nc.tensor.matmul(
                            ps[:],
                            w23t[:, w3off + k * H1 + m * 128:
                                 w3off + k * H1 + (m + 1) * 128],
                            ztile[:, k * T:(k + 1) * T],
                            start=(k == 0), stop=(k == KC3 - 1))
                    evict(dtile[:, m * T:(m + 1) * T], ps[:], relu=True)

                # L4: y[D_OUT, T] = W4.T @ d; two m-half stores on the scalar
                # HWDGE ring (short tail: hardware descriptor generation)
                ytile = yp.tile([128, MC4 * T], ydt, tag="y")
                yhalf = 128 * MH * T
                for m in range(MC4):
                    ps = ppb.tile([128, T], f32, tag="ps")
                    for k in range(KC4):
                        nc.tensor.matmul(
                            ps[:],
                            w4t[:, k * D_OUT + m * 128:k * D_OUT + (m + 1) * 128],
                            dtile[:, k * T:(k + 1) * T],
                            start=(k == 0), stop=(k == KC4 - 1))
                    evict(ytile[:, m * T:(m + 1) * T], ps[:], relu=False,
                          on_vector=(m % 2 == 0))
                    if m % MH == MH - 1:
                        h = m // MH
                        nc.scalar.dma_start(
                            yt[yo + h * yhalf:yo + (h + 1) * yhalf]
                            .rearrange("(p f) -> p f", p=128),
                            ytile[:, h * MH * T:(h + 1) * MH * T])

    nc.compile()
    return nc


def kernel(**inputs) -> np.ndarray:
    from concourse.bass_utils import run_bass_kernel_spmd

    X = np.ascontiguousarray(inputs["X"], dtype=np.float32)
    sym_ids = np.asarray(inputs["sym_ids"]).astype(np.int64).ravel()
    We = [inputs["We1"], inputs["We2"], inputs["Wd1"], inputs["Wd2"]]
    be = [np.asarray(inputs["be1"], dtype=np.float32),
          np.asarray(inputs["be2"], dtype=np.float32),
          np.asarray(inputs["bd1"], dtype=np.float32),
          np.asarray(inputs["bd2"], dtype=np.float32)]

    N, D_IN = X.shape
    E, _, H1 = We[0].shape
    LAT = We[1].shape[2]
    D_OUT = We[3].shape[2]
    KC1 = D_IN // 128
    MC4 = D_OUT // 128
    KH = KC1 // 2
    MH = MC4 // 2
    use_bias = any(np.any(b) for b in be)

    # ---- host routing: per-expert, per-core index assignment ----
    core_idx = [[None] * E for _ in range(N_CORES)]
    C_e = [0] * E
    for e in range(E):
        idx = np.flatnonzero(sym_ids == e)
        n = len(idx)
        base, rem = divmod(n, N_CORES)
        s = 0
        for c in range(N_CORES):
            cnt = base + (1 if c < rem else 0)
            core_idx[c][e] = idx[s:s + cnt]
            s += cnt
        C_e[e] = base + (1 if rem else 0)

    # per-expert column tiling: tiles of width <=512, multiples of 8
    tiles = []
    off = 0
    for e in range(E):
        ce = max(C_e[e], 1)
        nt = max(1, math.ceil(ce / 512))
        T = -(-math.ceil(ce / nt) // 8) * 8
        Ts = [T] * nt
        tiles.append((off, tuple(Ts)))
        off += sum(Ts)
    C_tot = off

    # ---- build / fetch compiled program ----
    dims = (D_IN, H1, LAT, D_OUT, E, C_tot)
    n_bias_cols = E * (H1 + LAT + H1 + D_OUT) // 128
    key = (dims, tuple(tiles), use_bias, _MODE, _Y_F16)
    nc = _PROGRAM_CACHE.get(key)
    if nc is None:
        nc = _build_program(dims, tiles, use_bias, n_bias_cols, _MODE)
        _PROGRAM_CACHE[key] = nc

    # ---- prepare inputs ----
    if _MODE == "f16":
        rnd = lambda a: np.ascontiguousarray(a, dtype=np.float32).astype(
            np.float16)
        mm_np = np.float16
    else:
        rnd = _round_fp32r
        mm_np = np.float32
    XrT = np.ascontiguousarray(rnd(X).T)                     # [D_IN, N]
    XrT_z = np.concatenate(
        [XrT, np.zeros((D_IN, 1), mm_np)], axis=1)           # pad col = N

    def wdev(w, kc, mw):
        return np.ascontiguousarray(
            rnd(w).reshape(E, kc, 128, mw).transpose(0, 2, 1, 3)
            .reshape(E, 128, kc * mw))

    W1d = wdev(We[0], KC1, H1)
    W23d = np.ascontiguousarray(np.concatenate(
        [wdev(We[1], H1 // 128, LAT), wdev(We[2], LAT // 128, H1)], axis=2))
    W4d = wdev(We[3], H1 // 128, D_OUT)

    seq, x_total, y_total = _plan(dims, tiles)

    bias_h = None
    if use_bias:
        bias_h = np.zeros((128, n_bias_cols), np.float32)
        col = 0
        for e in range(E):
            for b in (be[0][e], be[1][e], be[2][e], be[3][e]):
                for mch in range(len(b) // 128):
                    bias_h[:, col] = b[mch * 128:(mch + 1) * 128]
                    col += 1

    perms = []
    in_maps = []
    for c in range(N_CORES):
        perm = np.full(C_tot, N, dtype=np.int64)
        for e in range(E):
            o = tiles[e][0]
            idx = core_idx[c][e]
            perm[o:o + len(idx)] = idx
        perms.append(perm)
        g3 = XrT_z[:, perm].reshape(KC1, 128, C_tot)
        xflat = np.empty(x_total, dtype=mm_np)
        for si, (e, t, co, T, xo, yo) in enumerate(seq):
            if si == 0:
                # tile 0: two partition-major halves (big-packet prologue)
                half = 128 * KH * T
                xflat[xo:xo + half] = (
                    g3[:KH, :, co:co + T].transpose(1, 0, 2).reshape(-1))
                xflat[xo + half:xo + 2 * half] = (
                    g3[KH:, :, co:co + T].transpose(1, 0, 2).reshape(-1))
            else:
                # k-major chunks: each [128, T] chunk contiguous
                xflat[xo:xo + 128 * KC1 * T] = g3[:, :, co:co + T].reshape(-1)
        m = {"xt": xflat, "w1": W1d, "w23": W23d, "w4": W4d}
        if use_bias:
            m["bias"] = bias_h
        in_maps.append(m)

    res = run_bass_kernel_spmd(nc, in_maps, core_ids=list(range(N_CORES)),
                               trace=_TRACE)
    _LAST["res"] = res

    # ---- unshard ----
    Y = np.empty((N, D_OUT), dtype=np.float32)
    for c in range(N_CORES):
        yflat = np.asarray(res.results[c]["yt"], dtype=np.float32)
        ytc = np.empty((D_OUT, C_tot), dtype=np.float32)
        for e, t, co, T, xo, yo in seq:
            yhalf = 128 * MH * T
            for h in range(2):
                blk = (yflat[yo + h * yhalf:yo + (h + 1) * yhalf]
                       .reshape(128, MH, T).transpose(1, 0, 2)
                       .reshape(MH * 128, T))
                ytc[h * MH * 128:(h + 1) * MH * 128, co:co + T] = blk
        perm = perms[c]
        valid = perm != N
        Y[perm[valid]] = ytc.T[valid]
    return Y
